# revision 9
# baseline (speedup 1.0000x reference)
"""Trainium2 Bass kernel for MergedQKVParallelLinearWithLoRA.

Computes out = x @ W_qkv^T + b_qkv + per-token-LoRA, where each token t uses
adapter l_t = lora_indices[t]:
    shrink_s = x @ A_s[l_t]^T            (R=16 per slice s in {q,k,v})
    out[:, slice_s] += shrink_s @ B_s[l_t]^T

Strategy (8 NeuronCores, token-parallel):
  - Each core handles 1024 tokens, all 6144 output columns.
  - Host pre-transposes: xT [H, Tc] per core, wT [H, OUT], aT [H, 3*L*R],
    bT [L*R, OUT] (per-slice packed), plus a one-hot adapter mask expanded to
    [3*L*R, Tc] so LoRA becomes two dense matmuls (L=16 is small):
        shrinkT_all = aT^T @ x^T          [768, Tc]   (dense over adapters)
        shrinkT     = shrinkT_all * mask  (zero non-selected adapters)
        lora_out    = shrinkT_slice^T @ bT_slice  (accumulated in PSUM on top
                                                   of the base GEMM)
  - Matmuls run in bf16 (true full PE rate on HW; fp32/f32r stream at ~1/4
    rate on silicon even though the cost model says otherwise). Host casts
    x/w/a/b to bf16; PSUM accumulates fp32; bias added during the
    PSUM->SBUF copy on DVE. Norm rel-err vs fp32 reference ~2e-3, well
    inside the 2e-2 gate.
"""

import numpy as np

T = 8192
H = 4096
OUT_Q = 4096
OUT_KV = 1024
OUT = OUT_Q + 2 * OUT_KV  # 6144
L = 16
R = 16
LR3 = 3 * L * R  # 768
NCORES = 8
TC = T // NCORES  # 1024

_cache = {}


def _build(h, out_q, out_kv, tc_tokens, reps=1, timing_inputs=False, skip_lora=False, skip_main=False):
    """Build the per-core Bass program. All cores run the same NEFF (SPMD).

    reps > 1 wraps the whole body in a device-side For_i loop — used by the
    test harness to measure per-iteration HW time via wall-clock deltas.
    timing_inputs=True declares inputs as Internal DRAM (uninitialized, no
    host transfer) so wall-clock deltas are dominated by device exec time.
    """
    import concourse.bass as bass  # noqa: F401
    import concourse.mybir as mybir
    import concourse.tile as tile
    from concourse import bacc

    f32 = mybir.dt.float32
    bf16 = mybir.dt.bfloat16

    out_total = out_q + 2 * out_kv
    NH = h // 128          # contraction tiles
    NT = tc_tokens // 128  # token tiles (output partition dim)
    NOB = out_total // 512  # output column blocks
    NQB = out_q // 512      # q blocks
    NKB = out_kv // 512     # k blocks
    NC512 = tc_tokens // 512  # 512-token chunks for shrink
    NJ = LR3 // 128        # 6 lr tiles

    assert out_q % 512 == 0 and out_kv % 512 == 0 and tc_tokens % 512 == 0

    nc = bacc.Bacc(None, target_bir_lowering=False)

    in_kw = {} if timing_inputs else {"kind": "ExternalInput"}
    xT = nc.dram_tensor("xT", [h, tc_tokens], bf16, **in_kw)
    wT = nc.dram_tensor("wT", [h, out_total], bf16, **in_kw)
    aT = nc.dram_tensor("aT", [h, LR3], bf16, **in_kw)
    bT = nc.dram_tensor("bT", [2 * 128, out_total], bf16, **in_kw)
    maskT = nc.dram_tensor("maskT", [LR3, tc_tokens], f32, **in_kw)
    biasb = nc.dram_tensor("biasb", [128, out_total], f32, **in_kw)
    if timing_inputs:
        # keep the big result internal; expose only a tiny sink so per-call
        # host<->device transfer stays negligible for wall-delta timing
        out = nc.dram_tensor("out", [tc_tokens, out_total], f32)
        sink = nc.dram_tensor("sink", [128, 512], f32, kind="ExternalOutput")
    else:
        out = nc.dram_tensor(
            "out", [tc_tokens, out_total], f32, kind="ExternalOutput"
        )
        sink = None

    with tile.TileContext(nc) as tc:
        from contextlib import ExitStack

        with ExitStack() as ctx:
            xp = ctx.enter_context(tc.tile_pool(name="xp", bufs=1))
            sp = ctx.enter_context(tc.tile_pool(name="sp", bufs=1))
            pp = ctx.enter_context(tc.tile_pool(name="pp", bufs=8, space="PSUM"))
            atp = ctx.enter_context(tc.tile_pool(name="atp", bufs=2))
            mp = ctx.enter_context(tc.tile_pool(name="mp", bufs=2))
            wp = ctx.enter_context(tc.tile_pool(name="wp", bufs=4))
            btp = ctx.enter_context(tc.tile_pool(name="btp", bufs=3))
            bp2 = ctx.enter_context(tc.tile_pool(name="bp2", bufs=2))
            op = ctx.enter_context(tc.tile_pool(name="op", bufs=4))

            loop_ctx = tc.For_i(0, reps, 1) if reps > 1 else None
            if loop_ctx is not None:
                loop_ctx.__enter__()

            # Resident x^T: [128, NH, Tc] bf16 (partition = h % 128)
            xT_sb = xp.tile([128, NH, tc_tokens], bf16, name="xT_sb", tag="xT_sb")
            for a in range(NH):
                nc.sync.dma_start(
                    xT_sb[:, a, :], xT[a * 128:(a + 1) * 128, :]
                )
            # Resident masked shrink^T: [128, NJ, Tc]
            shrT = sp.tile([128, NJ, tc_tokens], bf16, name="shrT", tag="shrT")

            # ---- Phase 1: LoRA shrink (dense over adapters) + mask ----
            for th in range(NC512 if not skip_lora else 0):
                tsl = slice(th * 512, (th + 1) * 512)
                ps = [
                    pp.tile([128, 512], f32, name=f"shps_{th}_{j}", tag="ps")
                    for j in range(NJ)
                ]
                for hh in range(NH):
                    at = atp.tile([128, LR3], bf16, name=f"at_{th}_{hh}", tag="at")
                    nc.sync.dma_start(
                        at, aT[hh * 128:(hh + 1) * 128, :]
                    )
                    for j in range(NJ):
                        nc.tensor.matmul(
                            ps[j][:],
                            at[:, j * 128:(j + 1) * 128],
                            xT_sb[:, hh, tsl],
                            start=(hh == 0),
                            stop=(hh == NH - 1),
                        )
                for j in range(NJ):
                    m = mp.tile([128, 512], f32, name=f"m_{th}_{j}", tag="m")
                    nc.sync.dma_start(m, maskT[j * 128:(j + 1) * 128, tsl])
                    nc.vector.tensor_mul(shrT[:, j, tsl], ps[j][:], m[:])

            # ---- Phase 2: base GEMM + LoRA expand + bias ----
            for ob in range(NOB if not skip_main else 0):
                osl = slice(ob * 512, (ob + 1) * 512)
                # which slice (q/k/v) this 512-col block belongs to
                if ob < NQB:
                    jbase = 0
                elif ob < NQB + NKB:
                    jbase = 2
                else:
                    jbase = 4
                ps = [
                    pp.tile([128, 512], f32, name=f"mps_{ob}_{t}", tag="ps")
                    for t in range(NT)
                ]
                for hh in range(NH):
                    w = wp.tile([128, 512], bf16, name=f"w_{ob}_{hh}", tag="w")
                    nc.sync.dma_start(
                        w, wT[hh * 128:(hh + 1) * 128, osl]
                    )
                    for t in range(NT):
                        nc.tensor.matmul(
                            ps[t][:],
                            xT_sb[:, hh, t * 128:(t + 1) * 128],
                            w[:],
                            start=(hh == 0),
                            stop=(skip_lora and hh == NH - 1),
                        )
                for jj in range(2 if not skip_lora else 0):
                    bt = btp.tile([128, 512], bf16, name=f"bt_{ob}_{jj}", tag="bt")
                    nc.sync.dma_start(
                        bt, bT[jj * 128:(jj + 1) * 128, osl]
                    )
                    for t in range(NT):
                        nc.tensor.matmul(
                            ps[t][:],
                            shrT[:, jbase + jj, t * 128:(t + 1) * 128],
                            bt[:],
                            start=False,
                            stop=(jj == 1),
                        )
                bias_t = bp2.tile([128, 512], f32, name=f"bias_{ob}", tag="bias")
                nc.sync.dma_start(bias_t, biasb[:, osl])
                for t in range(NT):
                    o = op.tile([128, 512], f32, name=f"o_{ob}_{t}", tag="o")
                    nc.vector.tensor_add(o[:], ps[t][:], bias_t[:])
                    # out-stores go on the ACT HWDGE queue so they never
                    # head-block the SP queue's weight prefetch stream
                    nc.scalar.dma_start(out[t * 128:(t + 1) * 128, osl], o[:])

            if loop_ctx is not None:
                loop_ctx.__exit__(None, None, None)

            if sink is not None:
                nc.scalar.dma_start(sink[:], out[0:128, 0:512])

    nc.compile()
    return nc


def _get_nc(h=H, out_q=OUT_Q, out_kv=OUT_KV, tc_tokens=TC, reps=1,
            timing_inputs=False, skip_lora=False, skip_main=False):
    key = (h, out_q, out_kv, tc_tokens, reps, timing_inputs, skip_lora, skip_main)
    if key not in _cache:
        _cache[key] = _build(
            h, out_q, out_kv, tc_tokens, reps=reps, timing_inputs=timing_inputs,
            skip_lora=skip_lora, skip_main=skip_main,
        )
    return _cache[key]


def _host_prep(x, w_qkv, b_qkv, a_q, a_k, a_v, b_q, b_k, b_v, lora_indices,
               n_cores=NCORES):
    """Build per-core input maps (host-side transposes/packing/bf16 cast)."""
    import ml_dtypes

    f = np.float32
    bf = ml_dtypes.bfloat16
    x = np.ascontiguousarray(np.asarray(x, f).astype(bf))
    t_total, h = x.shape
    tc_tokens = t_total // n_cores
    out_q = np.asarray(b_q).shape[1]
    out_kv = np.asarray(b_k).shape[1]
    out_total = out_q + 2 * out_kv

    wT = np.ascontiguousarray(np.asarray(w_qkv, f).astype(bf).T)  # [H, OUT]
    l, r = np.asarray(a_q).shape[:2]
    aT = np.ascontiguousarray(
        np.concatenate(
            [np.asarray(a, f).reshape(l * r, h) for a in (a_q, a_k, a_v)], axis=0
        ).astype(bf).T
    )  # [H, 3*L*R]
    bT = np.ascontiguousarray(
        np.concatenate(
            [
                np.asarray(b, f).transpose(0, 2, 1).reshape(l * r, -1)
                for b in (b_q, b_k, b_v)
            ],
            axis=1,
        ).astype(bf)
    )  # [L*R, OUT]
    biasb = np.ascontiguousarray(
        np.broadcast_to(np.asarray(b_qkv, f), (128, out_total))
    )

    li = np.asarray(lora_indices).astype(np.int64)
    oh = (li[:, None] == np.arange(l)[None, :]).astype(f)       # [T, L]
    mask_exp = np.repeat(oh, r, axis=1)                          # [T, L*R]
    maskT_full = np.ascontiguousarray(np.tile(mask_exp.T, (3, 1)))  # [3LR, T]

    in_maps = []
    for c in range(n_cores):
        tsl = slice(c * tc_tokens, (c + 1) * tc_tokens)
        in_maps.append(
            {
                "xT": np.ascontiguousarray(x[tsl].T),
                "wT": wT,
                "aT": aT,
                "bT": bT,
                "maskT": np.ascontiguousarray(maskT_full[:, tsl]),
                "biasb": biasb,
            }
        )
    return in_maps


def kernel(x, w_qkv, b_qkv, a_q, a_k, a_v, b_q, b_k, b_v, lora_indices):
    from concourse.bass_utils import run_bass_kernel_spmd

    in_maps = _host_prep(
        x, w_qkv, b_qkv, a_q, a_k, a_v, b_q, b_k, b_v, lora_indices
    )
    nc = _get_nc()
    core_ids = list(range(NCORES))
    res = run_bass_kernel_spmd(nc, in_maps, core_ids)
    return np.concatenate([res.results[c]["out"] for c in core_ids], axis=0)



# revision 10
# speedup vs baseline: 1.3140x; 1.3140x over previous
"""Trainium2 Bass kernel for MergedQKVParallelLinearWithLoRA.

Computes out = x @ W_qkv^T + b_qkv + per-token-LoRA, where each token t uses
adapter l_t = lora_indices[t]:
    shrink_s = x @ A_s[l_t]^T            (R=16 per slice s in {q,k,v})
    out[:, slice_s] += shrink_s @ B_s[l_t]^T

Strategy (8 NeuronCores, token-parallel):
  - Each core handles 1024 tokens, all 6144 output columns.
  - Host pre-shuffles everything into partition-major [128, ...] layouts so
    each stream loads with a handful of large DMAs (HWDGE per-DMA fixed cost
    is what killed the small-tile version):
      xTs   [128, NH, Tc]   bf16  (x^T,   p = h % 128)   2 DMAs, resident
      wTs   [128, NH, OUT]  bf16  (W^T)                  2 DMAs per 512-col
      aTs   [128, NH, 3LR]  bf16  (A^T packed q|k|v)     8 DMAs per th chunk
      mTs   [128, NJ, Tc]   bf16  one-hot adapter mask   1 DMA per th chunk
      bTs   [128, 2, OUT]   bf16  (B^T, lr-major)        1 DMA, resident
      bias  [128, OUT]      bf16  broadcast bias         1 DMA, resident
      outS  [128, NT, OUT]  bf16  output, host unshuffles + casts to f32
  - LoRA becomes dense matmuls over all L adapters (L=16 small):
        shrinkT_all = aTs^T @ x^T       [768, Tc]
        shrinkT     = shrinkT_all * mask    (zero non-selected adapters)
        out_slice  += shrinkT_slice^T @ bTs_slice   (accumulated in PSUM)
  - All matmuls in bf16 (true full PE rate; fp32/f32r stream ~4x slower on
    silicon). PSUM accumulates fp32; bias added during PSUM->SBUF copy on
    DVE (writes bf16). Norm rel-err vs fp32 reference ~2e-3 (gate 2e-2).
"""

import numpy as np

T = 8192
H = 4096
OUT_Q = 4096
OUT_KV = 1024
OUT = OUT_Q + 2 * OUT_KV  # 6144
L = 16
R = 16
LR3 = 3 * L * R  # 768
NCORES = 8
TC = T // NCORES  # 1024

_cache = {}


def _build(h, out_q, out_kv, tc_tokens, reps=1, timing_inputs=False, skip_lora=False, skip_main=False):
    """Build the per-core Bass program. All cores run the same NEFF (SPMD).

    reps > 1 wraps the whole body in a device-side For_i loop — used by the
    test harness to measure per-iteration HW time via wall-clock deltas.
    timing_inputs=True declares inputs as Internal DRAM (uninitialized, no
    host transfer) so wall-clock deltas are dominated by device exec time.
    """
    import concourse.bass as bass  # noqa: F401
    import concourse.mybir as mybir
    import concourse.tile as tile
    from concourse import bacc

    f32 = mybir.dt.float32
    bf16 = mybir.dt.bfloat16

    out_total = out_q + 2 * out_kv
    NH = h // 128          # contraction tiles
    NT = tc_tokens // 128  # token tiles (output partition dim)
    NOB = out_total // 512  # output column blocks
    NQB = out_q // 512      # q blocks
    NKB = out_kv // 512     # k blocks
    NC512 = tc_tokens // 512  # 512-token chunks for shrink
    NJ = LR3 // 128        # 6 lr tiles

    assert out_q % 512 == 0 and out_kv % 512 == 0 and tc_tokens % 512 == 0

    nc = bacc.Bacc(None, target_bir_lowering=False)

    in_kw = {} if timing_inputs else {"kind": "ExternalInput"}
    xTs = nc.dram_tensor("xTs", [128, NH, tc_tokens], bf16, **in_kw)
    wTs = nc.dram_tensor("wTs", [128, NH, out_total], bf16, **in_kw)
    aTs = nc.dram_tensor("aTs", [128, NH, LR3], bf16, **in_kw)
    bTs = nc.dram_tensor("bTs", [128, 2, out_total], bf16, **in_kw)
    mTs = nc.dram_tensor("mTs", [128, NJ, tc_tokens], bf16, **in_kw)
    biasb = nc.dram_tensor("biasb", [128, out_total], bf16, **in_kw)
    if timing_inputs:
        # keep the big result internal; expose only a tiny sink so per-call
        # host<->device transfer stays negligible for wall-delta timing
        outS = nc.dram_tensor("outS", [128, NT, out_total], bf16)
        sink = nc.dram_tensor("sink", [128, 512], bf16, kind="ExternalOutput")
    else:
        outS = nc.dram_tensor(
            "outS", [128, NT, out_total], bf16, kind="ExternalOutput"
        )
        sink = None

    with tile.TileContext(nc) as tc:
        from contextlib import ExitStack

        with ExitStack() as ctx:
            xp = ctx.enter_context(tc.tile_pool(name="xp", bufs=1))
            sp = ctx.enter_context(tc.tile_pool(name="sp", bufs=1))
            rp = ctx.enter_context(tc.tile_pool(name="rp", bufs=1))
            pp = ctx.enter_context(tc.tile_pool(name="pp", bufs=8, space="PSUM"))
            atp = ctx.enter_context(tc.tile_pool(name="atp", bufs=2))
            mp = ctx.enter_context(tc.tile_pool(name="mp", bufs=2))
            wp = ctx.enter_context(tc.tile_pool(name="wp", bufs=2))
            op = ctx.enter_context(tc.tile_pool(name="op", bufs=2))

            loop_ctx = tc.For_i(0, reps, 1) if reps > 1 else None
            if loop_ctx is not None:
                loop_ctx.__enter__()

            # Resident tensors: x^T, masked shrink^T, B^T, bias
            xT_sb = xp.tile([128, NH, tc_tokens], bf16, name="xT_sb", tag="xT_sb")
            for g in range(2):
                nc.sync.dma_start(
                    xT_sb[:, g * NH // 2:(g + 1) * NH // 2, :],
                    xTs[:, g * NH // 2:(g + 1) * NH // 2, :],
                )
            shrT = sp.tile([128, NJ, tc_tokens], bf16, name="shrT", tag="shrT")
            bt_sb = rp.tile([128, 2, out_total], bf16, name="bt_sb", tag="bt_sb")
            nc.scalar.dma_start(bt_sb, bTs[:, :, :])
            bias_sb = rp.tile([128, out_total], bf16, name="bias_sb", tag="bias_sb")
            nc.scalar.dma_start(bias_sb, biasb[:, :])

            # ---- Phase 1: LoRA shrink (dense over adapters) + mask ----
            for th in range(NC512 if not skip_lora else 0):
                tsl = slice(th * 512, (th + 1) * 512)
                ps = [
                    pp.tile([128, 512], f32, name=f"shps_{th}_{j}", tag="ps")
                    for j in range(NJ)
                ]
                for hg in range(NH // 4):
                    at = atp.tile([128, 4, LR3], bf16, name=f"at_{th}_{hg}", tag="at")
                    nc.sync.dma_start(at, aTs[:, hg * 4:(hg + 1) * 4, :])
                    for hh in range(4):
                        for j in range(NJ):
                            nc.tensor.matmul(
                                ps[j][:],
                                at[:, hh, j * 128:(j + 1) * 128],
                                xT_sb[:, hg * 4 + hh, tsl],
                                start=(hg == 0 and hh == 0),
                                stop=(hg == NH // 4 - 1 and hh == 3),
                            )
                m = mp.tile([128, NJ, 512], bf16, name=f"m_{th}", tag="m")
                nc.sync.dma_start(m, mTs[:, :, tsl])
                for j in range(NJ):
                    nc.vector.tensor_mul(shrT[:, j, tsl], ps[j][:], m[:, j, :])

            # ---- Phase 2: base GEMM + LoRA expand + bias ----
            for ob in range(NOB if not skip_main else 0):
                osl = slice(ob * 512, (ob + 1) * 512)
                # which slice (q/k/v) this 512-col block belongs to
                if ob < NQB:
                    jbase = 0
                elif ob < NQB + NKB:
                    jbase = 2
                else:
                    jbase = 4
                ps = [
                    pp.tile([128, 512], f32, name=f"mps_{ob}_{t}", tag="ps")
                    for t in range(NT)
                ]
                for wg in range(2):
                    w = wp.tile([128, NH // 2, 512], bf16, name=f"w_{ob}_{wg}", tag="w")
                    nc.sync.dma_start(
                        w, wTs[:, wg * NH // 2:(wg + 1) * NH // 2, osl]
                    )
                    for hh in range(NH // 2):
                        ah = wg * NH // 2 + hh
                        for t in range(NT):
                            nc.tensor.matmul(
                                ps[t][:],
                                xT_sb[:, ah, t * 128:(t + 1) * 128],
                                w[:, hh, :],
                                start=(ah == 0),
                                stop=(skip_lora and ah == NH - 1),
                            )
                for jj in range(2 if not skip_lora else 0):
                    for t in range(NT):
                        nc.tensor.matmul(
                            ps[t][:],
                            shrT[:, jbase + jj, t * 128:(t + 1) * 128],
                            bt_sb[:, jj, osl],
                            start=False,
                            stop=(jj == 1),
                        )
                o = op.tile([128, NT, 512], bf16, name=f"o_{ob}", tag="o")
                for t in range(NT):
                    nc.vector.tensor_add(o[:, t, :], ps[t][:], bias_sb[:, osl])
                # out-stores go on the ACT HWDGE queue so they never
                # head-block the SP queue's weight prefetch stream
                nc.scalar.dma_start(outS[:, :, osl], o[:])

            if loop_ctx is not None:
                loop_ctx.__exit__(None, None, None)

            if sink is not None:
                nc.scalar.dma_start(sink[:], outS[:, 0, 0:512])

    nc.compile()
    return nc


def _get_nc(h=H, out_q=OUT_Q, out_kv=OUT_KV, tc_tokens=TC, reps=1,
            timing_inputs=False, skip_lora=False, skip_main=False):
    key = (h, out_q, out_kv, tc_tokens, reps, timing_inputs, skip_lora, skip_main)
    if key not in _cache:
        _cache[key] = _build(
            h, out_q, out_kv, tc_tokens, reps=reps, timing_inputs=timing_inputs,
            skip_lora=skip_lora, skip_main=skip_main,
        )
    return _cache[key]


def _shuf(arr2d, p=128):
    """[K*p, N] -> [p, K, N] with out[q, k, n] = arr2d[k*p + q, n]."""
    k = arr2d.shape[0] // p
    return np.ascontiguousarray(arr2d.reshape(k, p, -1).transpose(1, 0, 2))


def _host_prep(x, w_qkv, b_qkv, a_q, a_k, a_v, b_q, b_k, b_v, lora_indices,
               n_cores=NCORES):
    """Build per-core input maps (host-side shuffles/packing/bf16 cast)."""
    import ml_dtypes

    f = np.float32
    bf = ml_dtypes.bfloat16
    x = np.asarray(x, f).astype(bf)
    t_total, h = x.shape
    tc_tokens = t_total // n_cores
    out_q = np.asarray(b_q).shape[1]
    out_kv = np.asarray(b_k).shape[1]
    out_total = out_q + 2 * out_kv

    wTs = _shuf(np.ascontiguousarray(np.asarray(w_qkv, f).astype(bf).T))
    l, r = np.asarray(a_q).shape[:2]
    aTs = _shuf(np.ascontiguousarray(
        np.concatenate(
            [np.asarray(a, f).reshape(l * r, h) for a in (a_q, a_k, a_v)], axis=0
        ).astype(bf).T
    ))
    bTs = _shuf(np.concatenate(
        [
            np.asarray(b, f).transpose(0, 2, 1).reshape(l * r, -1)
            for b in (b_q, b_k, b_v)
        ],
        axis=1,
    ).astype(bf))  # [128, 2, OUT]
    biasb = np.ascontiguousarray(
        np.broadcast_to(np.asarray(b_qkv, f).astype(bf), (128, out_total))
    )

    li = np.asarray(lora_indices).astype(np.int64)
    oh = (li[:, None] == np.arange(l)[None, :]).astype(f)       # [T, L]
    mask_exp = np.repeat(oh, r, axis=1)                          # [T, L*R]
    maskT_full = np.tile(mask_exp.T, (3, 1)).astype(bf)          # [3LR, T]

    in_maps = []
    for c in range(n_cores):
        tsl = slice(c * tc_tokens, (c + 1) * tc_tokens)
        in_maps.append(
            {
                "xTs": _shuf(np.ascontiguousarray(x[tsl].T)),
                "wTs": wTs,
                "aTs": aTs,
                "bTs": bTs,
                "mTs": _shuf(np.ascontiguousarray(maskT_full[:, tsl])),
                "biasb": biasb,
            }
        )
    return in_maps


def kernel(x, w_qkv, b_qkv, a_q, a_k, a_v, b_q, b_k, b_v, lora_indices):
    from concourse.bass_utils import run_bass_kernel_spmd

    in_maps = _host_prep(
        x, w_qkv, b_qkv, a_q, a_k, a_v, b_q, b_k, b_v, lora_indices
    )
    nc = _get_nc()
    core_ids = list(range(NCORES))
    res = run_bass_kernel_spmd(nc, in_maps, core_ids)
    tc_tokens = T // NCORES
    nt = tc_tokens // 128
    parts = []
    for c in core_ids:
        o = np.asarray(res.results[c]["outS"])  # [128, NT, OUT] bf16
        parts.append(o.transpose(1, 0, 2).reshape(tc_tokens, OUT).astype(np.float32))
    return np.concatenate(parts, axis=0)
